# revision 59
# baseline (speedup 1.0000x reference)
# Trainium2 Bass kernel for nn_Block_SA (dense_cnn self-attention block).
#
# Per-sample computation (C=64 channels, 64x64 spatial, N=4096 positions):
#   v   = relu(bn1(conv1x1(x)))                      # V for attention
#   s   = (x^T x) / sqrt(C)                          # [N, N] scores, Q=K=x
#   p   = softmax(s, axis=-1)
#   a   = V p^T  (a[d,n] = sum_m p[n,m] V[d,m])
#   z   = relu(bn2(depthwise3x3(a)))
#   out = bn3(conv1x1(z)) + x
#
# Distribution: batch B=8, one sample per NeuronCore (data parallel, no
# collectives). BN params are folded into conv weights on the host.
#
# On-chip algorithm (per core):
#   - Scores are computed TRANSPOSED: sT[m, n] tiles via matmul(lhsT=x[:,mtile],
#     rhs=x[:,nchunk]) so softmax's sum over m becomes a matmul reduction.
#     The K=64 contraction uses only half the PE array, so score matmuls are
#     row-packed two-at-a-time with tile_position (x duplicated on partitions
#     64-127) for ~2x PE throughput.
#   - Score/attention matmul operands are bf16 (PSUM accumulation stays fp32).
#     fp32-mode matmuls trip the HW activity throttle (util limit 0.5 for
#     ~half the run in the fp32r version); bf16 avoids it. Accuracy loss is
#     ~0.5% rel, well under the 2e-2 gate.
#   - exp() without max subtraction (scores/8 are small; fp32 is safe), in
#     batches of 3 m-tiles (3 PSUM banks) to amortize the ACT engine's fixed
#     per-instruction access latency. The scalar engine is the bottleneck
#     (~134us of exp at 1 elem/cycle/partition @1.2GHz) and runs ~97% busy
#     in steady state.
#   - The denominator sum_m exp(sT[m,n]) is obtained for free by augmenting
#     V^T with a ones column (row 64 of the AV accumulator).
#   - AV accumulates over all 32 m-tiles into one PSUM bank as serial K=128
#     matmuls: same-context streams tail-overlap (start-to-start ~215ns),
#     which beats row-packing them (two busy contexts cannot hide the fixed
#     ~173ns SBUF access latency).
#   - Normalization: 1/den via the fast custom-DVE reciprocal (18 bits; its
#     input must be partition-0-aligned — custom-DVE ops break on shifted
#     partition bases), broadcast across partitions on the (mostly idle)
#     Pool engine, multiply on DVE.
#   - Software-pipelined emission: each group's AV matmuls are emitted TWO
#     groups late, and each chunk's depthwise/conv3 are deferred into the
#     next chunk's group loop, so the in-order PE queue always has score
#     work while exp/normalize chains run on other engines.
#   - Depthwise 3x3 runs on the DVE as 9 scalar_tensor_tensor accumulation
#     taps over shifted 2-D window views of y (per-channel weights are
#     per-partition scalars). The LAST chunk's taps run as PE diag-weight
#     matmuls instead (split PE/DVE) to shorten the serial tail.
#   - conv3 + bias via augmented ones row; residual add in fp32; DMA out.
#   - bf16 ones rows arrive via DMA (single-partition wide memsets cost
#     ~3.5us on an engine).

import numpy as np

_EPS = 1e-5
_C = 64
_CP1 = 65
_N = 4096
_CH = 512          # free-dim chunk (one PSUM bank of fp32)
_NCH = _N // _CH   # 8 chunks
_MT = 128          # m-tile (partition dim of transposed score tiles)
_NMT = _N // _MT   # 32 m-tiles
_W = 64            # image width
# m-tiles per exp batch (3 PSUM banks max). The 2-tile group sits mid-chunk
# and its exp runs via the Schraudolph bit-trick (~2% element error, diluted
# to ~0.05% output error by softmax): DVE does the PSUM-side scale+bias,
# the (idle) Pool engine the int32 convert + bf16 cast — relieving the
# saturated scalar engine by ~1.1us per chunk.
_GROUPS = [3, 3, 3, 3, 2, 3, 3, 3, 3, 3, 3]
_SCHRA_A = 12102203.16 * 0.125
_SCHRA_B = float(127 * 2 ** 23 - 361007)
_NCONST = 138 + 9 * 64 + 9  # w1aug | w3aug | w2p | b2p | diags | -w2p

_STATE = {}


def _build_program(reps=1):
    import concourse.bacc as bacc
    import concourse.tile as tile
    from concourse import mybir

    F32 = mybir.dt.float32
    F32R = mybir.dt.float32r
    BF16 = mybir.dt.bfloat16
    U32 = mybir.dt.uint32
    AF = mybir.ActivationFunctionType
    ALU = mybir.AluOpType
    ONE_BITS = 0x3F800000

    nc = bacc.Bacc(None)

    xd = nc.dram_tensor("x", [_C, _N], F32, kind="ExternalInput")
    # packed weights -> one DMA: cols 0:64 w1aug, 64:128 w3aug,
    # 128:137 w2p (rows 0:64), 137 b2p, 138:714 diag(w2p[:,k]) k=0..8
    cd = nc.dram_tensor("consts", [_CP1, _NCONST], F32, kind="ExternalInput")
    onesd = nc.dram_tensor("ones_bf", [1, _N], BF16, kind="ExternalInput")
    outd = nc.dram_tensor("out", [_C, _N], F32, kind="ExternalOutput")

    with tile.TileContext(nc) as tc:
        with (
            tc.tile_pool(name="persist", bufs=1) as pp,
            tc.tile_pool(name="small", bufs=2) as sp,
            tc.tile_pool(name="pt_pool", bufs=4) as ptp,
            tc.tile_pool(name="ps_pool", bufs=2, space="PSUM") as psp,
            tc.tile_pool(name="po_pool", bufs=1, space="PSUM") as pop,
            tc.tile_pool(name="aux_pool", bufs=1, space="PSUM") as auxp,
        ):
            def emit_all():
                # ---- input staging. x is DMA'd from HBM once (fp32, kept for
                # the residual), cast to bf16 on DVE/Pool, and the bf16 copy is
                # duplicated to partitions 64:128 by SBUF-to-SBUF DMA so score
                # matmuls can row-pack.
                xo = pp.tile([_C, _N], F32, name="xo", tag="xo")
                xa = pp.tile([_CP1, _N], BF16, name="xa", tag="xa")
                xb2 = pp.tile([_MT, _N], BF16, name="xb2", tag="xb2")
                lo = 0
                for s, w in enumerate([512, 512, 1024, 1024, 1024]):
                    sl = slice(lo, lo + w)
                    lo += w
                    nc.sync.dma_start(xo[:, sl], xd[:, sl])
                    eng = nc.vector if s < 4 else nc.gpsimd
                    eng.tensor_copy(xa[0:_C, sl], xo[:, sl])
                    nc.sync.dma_start(xb2[_C:_MT, sl], xa[0:_C, sl])
                # bf16 ones rows come in via DMA (a single-partition 4096-wide
                # memset costs ~3.5us on an engine; the DMA is ~free)
                nc.sync.dma_start(xa[_C:_CP1, :], onesd[:])

                # PE p-state warm-up: the tensor engine needs ~3us of
                # continuous execution to reach full clock, and it is idle
                # during the startup DMA wait anyway. Stream a few dummy
                # matmuls from a memset tile so chunk 0's real matmuls run
                # at full speed.
                wu = pp.tile([_C, _CH], BF16, name="wu", tag="wu")
                nc.vector.memset(wu[:], 0.5)
                wps = auxp.tile([_MT, _CH], F32, name="wps", tag="aux")
                for _ in range(10):
                    nc.tensor.matmul(
                        wps[0:_C, :], lhsT=wu[:, 0:_C], rhs=wu[:],
                        start=True, stop=True,
                    )

                cs = pp.tile([_CP1, _NCONST], F32, name="cs", tag="cs")
                nc.scalar.dma_start(cs[:], cd[:])
                w2s = cs[0:_C, 128:137]
                b2s = cs[0:_C, 137:138]
                w2n = cs[0:_C, 714:723]

                # rounded weight copies (bf16 everywhere on the PE)
                w1b = pp.tile([_CP1, _C], BF16, name="w1b", tag="w1b")
                nc.vector.tensor_copy(w1b[:], cs[:, 0:64])
                w3b = pp.tile([_CP1, _C], BF16, name="w3b", tag="w3b")
                nc.vector.tensor_copy(w3b[:], cs[:, 64:128])
                dgb = pp.tile([_C, 9 * _C], BF16, name="dgb", tag="dgb")
                nc.gpsimd.tensor_copy(dgb[:], cs[0:_C, 138:138 + 9 * _C])

                # V^T blocks: per m-tile a [128, 65] block (col 64 = ones)
                vt = pp.tile([_MT, _NMT * _CP1], BF16, name="vt", tag="vt")
                vt3 = vt.rearrange("p (t c) -> p t c", c=_CP1)
                nc.gpsimd.memset(vt3[:, :, _C:_CP1], 1.0)

                # normalized attention output (bf16: feeds the PE depthwise).
                # One zeroed pad row of 64 on each side so flat row-spanning
                # shifted reads stay in bounds.
                yrp = pp.tile([_C, _N + 2 * _W], BF16, name="yrp", tag="yrp")
                nc.gpsimd.memset(yrp[:, 0:_W], 0.0)
                nc.gpsimd.memset(yrp[:, _W + _N : _N + 2 * _W], 0.0)
                yr = yrp[:, _W : _W + _N]
                # post-depthwise activations (+ones row) feeding conv3
                zr = pp.tile([_CP1, _N], BF16, name="zr", tag="zr")
                nc.sync.dma_start(zr[_C:_CP1, :], onesd[:])
                zrv = zr[0:_C, :].rearrange("c (h w) -> c h w", w=_W)

                # ---- V^T groups: emitted lazily (interleaved into chunk 0's
                # group loop) so the cold PE isn't blocked on them at startup.
                # relu on DVE (not ACT) so the scalar engine runs Exp only.
                _vt_emitted = [0]

                def emit_vt_groups(need_mtiles):
                    while _vt_emitted[0] * 8 < need_mtiles:
                        g = _vt_emitted[0]
                        vps = auxp.tile([_MT, 8 * _C], F32, name="vps", tag="aux")
                        for j in range(8):
                            m = 8 * g + j
                            nc.tensor.matmul(
                                vps[:, _C * j : _C * (j + 1)],
                                lhsT=xa[:, _MT * m : _MT * (m + 1)],
                                rhs=w1b[:],
                                start=True,
                                stop=True,
                            )
                        nc.vector.tensor_relu(
                            vt3[:, 8 * g : 8 * (g + 1), 0:_C],
                            vps[:].rearrange("p (t c) -> p t c", c=_C),
                        )
                        _vt_emitted[0] += 1

                # ---- depthwise 3x3 on the (otherwise idle) GpSimd engine:
                # per-channel tap weights are per-partition scalars, so each
                # tap is one scalar_tensor_tensor accumulation over a shifted
                # 2-D window view of y. No PE involvement, no wrap fix-ups.
                yrp3 = yrp.rearrange("c (h w) -> c h w", w=_W)  # row i = y row i-1

                def emit_dw(h0, h1):
                    nh = h1 - h0
                    za = sp.tile([_C, nh * _W], F32, name="za", tag="za", bufs=2)
                    za3 = za.rearrange("c (h w) -> c h w", w=_W)
                    # init with the center tap + bias: z = y*w4 + b
                    nc.vector.tensor_scalar(
                        za3[:], yrp3[:, h0 + 1 : h1 + 1, :], w2s[:, 4:5], b2s,
                        op0=ALU.mult, op1=ALU.add,
                    )
                    for k in [0, 1, 2, 3, 5, 6, 7, 8]:
                        dy, dx = k // 3 - 1, k % 3 - 1
                        hh0, hh1 = max(h0, -dy), min(h1, _W - dy)
                        if hh1 <= hh0:
                            continue
                        x0, x1 = max(0, -dx), _W - max(0, dx)
                        dst = za3[:, hh0 - h0 : hh1 - h0, x0:x1]
                        src = yrp3[:, hh0 + dy + 1 : hh1 + dy + 1, x0 + dx : x1 + dx]
                        nc.vector.scalar_tensor_tensor(
                            dst, src, w2s[:, k : k + 1], dst,
                            op0=ALU.mult, op1=ALU.add,
                        )
                    nc.vector.tensor_scalar(
                        zrv[:, h0:h1, :], za3[:], 0.0, 0.0, op0=ALU.max, op1=ALU.max
                    )

                def emit_dw_pe(h0, h1):
                    # PE variant (diag-weight matmuls over clipped 2-D window
                    # views): used for the LAST chunk, where the serial DVE
                    # chain would sit alone in the tail while the PE idles.
                    nh = h1 - h0
                    dwp = auxp.tile([_C, nh * _W], F32, name="dwp", tag="aux")
                    dwp3 = dwp.rearrange("c (h w) -> c h w", w=_W)
                    taps = []
                    for k in [4, 0, 1, 2, 3, 5, 6, 7, 8]:
                        dy, dx = k // 3 - 1, k % 3 - 1
                        hh0, hh1 = max(h0, -dy), min(h1, _W - dy)
                        if hh1 <= hh0:
                            continue
                        x0, x1 = max(0, -dx), _W - max(0, dx)
                        taps.append((k, hh0, hh1, x0, x1, dy, dx))
                    for i, (k, hh0, hh1, x0, x1, dy, dx) in enumerate(taps):
                        nc.tensor.matmul(
                            dwp3[:, hh0 - h0 : hh1 - h0, x0:x1],
                            lhsT=dgb[:, _C * k : _C * (k + 1)],
                            rhs=yrp3[:, hh0 + dy + 1 : hh1 + dy + 1,
                                     x0 + dx : x1 + dx],
                            start=(i == 0),
                            stop=(i == len(taps) - 1),
                            skip_group_check=True,
                        )
                    nc.vector.tensor_scalar(
                        zrv[:, h0:h1, :], dwp3[:], b2s, 0.0,
                        op0=ALU.add, op1=ALU.max,
                    )

                def emit_conv3(c):
                    # conv3 (+bias via ones row) + residual + store
                    pc = auxp.tile([_C, _CH], F32, name="pc", tag="aux")
                    nc.tensor.matmul(
                        pc[:],
                        lhsT=w3b[:],
                        rhs=zr[:, _CH * c : _CH * (c + 1)],
                        start=True,
                        stop=True,
                    )
                    outt = sp.tile([_C, _CH], F32, name="outt", tag="outt", bufs=2)
                    nc.vector.tensor_tensor(
                        outt[:], pc[:], xo[:, _CH * c : _CH * (c + 1)], op=ALU.add
                    )
                    nc.sync.dma_start(outd[:, _CH * c : _CH * (c + 1)], outt[:])

                # ---- main fused-attention loop over n-chunks ----
                # Deferred depthwise/conv3 closures, popped between the NEXT
                # chunk's score groups (keeps the in-order PE queue stall-free).
                pending = []
                # Software-pipelined emission: exp feeds an AV-group QUEUE
                # drained TWO groups behind the scores — the in-order PE queue
                # then always has score groups ready to run while the scalar
                # engine computes exp. The queue persists across chunk
                # boundaries; each chunk's normalize is emitted right after
                # its LAST AV group pops (so the accumulator is fully written
                # in emission order before being read).
                av_q = []
                _AV_DELAY = 2

                def emit_normalize(po, ci):
                    # normalize: y = u[0:64] * (1/u[64]). Custom-DVE ops need a
                    # partition-0-aligned source, so first stage the den row to
                    # partition 0 with a plain (shift-capable) DVE copy, then
                    # the fast reciprocal; partition-broadcast on Pool (PE and
                    # ACT stay out of this chain entirely).
                    dsb = sp.tile([1, _CH], F32, name="dsb", tag="dsb", bufs=2)
                    nc.vector.tensor_copy(dsb[:], po[_C : _C + 1, :])
                    invf = sp.tile([1, _CH], F32, name="invf", tag="invf", bufs=2)
                    nc.vector.reciprocal_approx_fast(out=invf[:], in_=dsb[:])
                    bcps = sp.tile([_C, _CH], F32, name="bcps", tag="bcps", bufs=2)
                    nc.gpsimd.partition_broadcast(bcps[:], invf[:])
                    nc.vector.tensor_tensor(
                        yr[:, _CH * ci : _CH * (ci + 1)], po[0:_C, :], bcps[:],
                        op=ALU.mult,
                    )
                    # queue this chunk's depthwise (and finish chunk ci-1:
                    # its boundary row needed this chunk's y). The last chunk's
                    # depthwise runs on the PE (tail latency).
                    if ci == _NCH - 1:
                        # split the tail depthwise across PE and DVE so the
                        # two halves run concurrently
                        def dwtail(ci=ci):
                            emit_dw_pe(8 * ci, 8 * ci + 4)
                            emit_dw(8 * ci + 4, 8 * ci + 7)
                        pending.append(dwtail)
                        dwf = emit_dw_pe
                    else:
                        dwf = emit_dw
                        pending.append(lambda f=dwf, ci=ci: f(8 * ci, 8 * ci + 7))
                    if ci >= 1:
                        def fin(ci=ci, f=dwf):
                            f(8 * ci - 1, 8 * ci)
                            emit_conv3(ci - 1)
                        pending.append(fin)

                def pop_av():
                    emit, need, fin_ci_po = av_q.pop(0)
                    # V^T blocks are emitted just-in-time for the AV group
                    # that consumes them (eager emission would delay the
                    # early score groups and starve exp)
                    if need is not None:
                        emit_vt_groups(need)
                    emit()
                    if fin_ci_po is not None:
                        emit_normalize(*fin_ci_po)

                for ci in range(_NCH):
                    po = pop.tile([_MT, _CH], F32, name="po", tag="po")
                    m = 0
                    # chunk 0 leads with small groups so the scalar engine's exp
                    # stream starts as soon as the first score tile exists
                    groups = ([1, 2] + [3] * 9 + [2]) if ci == 0 else _GROUPS
                    for gi, msz in enumerate(groups):
                        ps = psp.tile([_MT, _CH * msz], F32, name="ps", tag="ps")
                        for j in range(msz):
                            mt = m + j
                            if mt % 2 == 0:
                                src, rows, tp = xa, slice(0, _C), (0, 0)
                            else:
                                src, rows, tp = xb2, slice(_C, _MT), (_C, 0)
                            nc.tensor.matmul(
                                ps[:, _CH * j : _CH * (j + 1)],
                                lhsT=src[rows, _MT * mt : _MT * (mt + 1)],
                                rhs=src[rows, _CH * ci : _CH * (ci + 1)],
                                start=True,
                                stop=True,
                                tile_position=tp,
                            )
                        pt = ptp.tile([_MT, _CH * msz], BF16, name="pt", tag="pt")
                        if ci >= 1 and msz == 2:
                            # bit-trick exp: DVE reads PSUM (Pool cannot),
                            # Pool does the SBUF-side converts
                            tfx = sp.tile([_MT, 2 * _CH], F32, name="tfx",
                                          tag="tfx", bufs=2)
                            nc.vector.tensor_scalar(
                                tfx[:], ps[:], _SCHRA_A, _SCHRA_B,
                                op0=ALU.mult, op1=ALU.add,
                            )
                            tix = sp.tile([_MT, 2 * _CH], mybir.dt.int32,
                                          name="tix", tag="tix", bufs=2)
                            nc.gpsimd.tensor_copy(tix[:], tfx[:])
                            nc.gpsimd.tensor_copy(pt[:], tix[:].bitcast(F32))
                        else:
                            nc.scalar.activation(pt[:], ps[:], AF.Exp,
                                                 scale=0.125)

                        def av_group(po=po, pt=pt, m=m, msz=msz):
                            for j in range(msz):
                                nc.tensor.matmul(
                                    po[0:_CP1, :],
                                    lhsT=vt[:, _CP1 * (m + j) : _CP1 * (m + j + 1)],
                                    rhs=pt[:, _CH * j : _CH * (j + 1)],
                                    start=(m + j == 0),
                                    stop=(m + j == _NMT - 1),
                                    skip_group_check=True,
                                )

                        last = m + msz == _NMT
                        av_q.append((av_group, (m + msz) if ci == 0 else None,
                                     (po, ci) if last else None))
                        while len(av_q) > _AV_DELAY:
                            pop_av()
                        m += msz
                        if gi in (5, 10) and pending:
                            pending.pop(0)()
                while av_q:
                    pop_av()
                for f in pending:
                    f()
                emit_dw_pe(_N // _W - 1, _N // _W)  # last row (no dy=+1 tap)
                emit_conv3(_NCH - 1)

            if reps == 1:
                emit_all()
            else:
                with tc.For_i(0, reps, 1):
                    emit_all()

    nc.finalize()
    return nc


def _get_nc():
    if "nc" not in _STATE:
        _STATE["nc"] = _build_program()
    return _STATE["nc"]


def _prep_inputs(x, w1, bn1_g, bn1_b, bn1_m, bn1_v,
                 w2, bn2_g, bn2_b, bn2_m, bn2_v,
                 w3, bn3_g, bn3_b, bn3_m, bn3_v):
    f32 = np.float32
    x = np.asarray(x, f32)
    inv1 = np.asarray(bn1_g, f32) / np.sqrt(np.asarray(bn1_v, f32) + _EPS)
    w1p = np.asarray(w1, f32)[:, :, 0, 0] * inv1[:, None]
    b1p = np.asarray(bn1_b, f32) - np.asarray(bn1_m, f32) * inv1
    w1aug = np.concatenate([w1p.T, b1p[None, :]], axis=0)

    inv2 = np.asarray(bn2_g, f32) / np.sqrt(np.asarray(bn2_v, f32) + _EPS)
    w2p = np.asarray(w2, f32)[:, 0].reshape(_C, 9) * inv2[:, None]
    b2p = (np.asarray(bn2_b, f32) - np.asarray(bn2_m, f32) * inv2)[:, None]

    inv3 = np.asarray(bn3_g, f32) / np.sqrt(np.asarray(bn3_v, f32) + _EPS)
    w3p = np.asarray(w3, f32)[:, :, 0, 0] * inv3[:, None]
    b3p = np.asarray(bn3_b, f32) - np.asarray(bn3_m, f32) * inv3
    w3aug = np.concatenate([w3p.T, b3p[None, :]], axis=0)

    consts = np.zeros((_CP1, _NCONST), f32)
    consts[:, 0:64] = w1aug
    consts[:, 64:128] = w3aug
    consts[0:_C, 128:137] = w2p
    consts[0:_C, 137:138] = b2p
    for k in range(9):
        consts[0:_C, 138 + _C * k : 138 + _C * (k + 1)] = np.diag(w2p[:, k])
    consts[0:_C, 714:723] = -w2p

    import ml_dtypes
    ones_bf = np.ones((1, _N), dtype=ml_dtypes.bfloat16)
    B = x.shape[0]
    in_maps = []
    for i in range(B):
        in_maps.append({
            "x": np.ascontiguousarray(x[i].reshape(_C, _N)),
            "consts": consts,
            "ones_bf": ones_bf,
        })
    return in_maps


def kernel(**inputs) -> np.ndarray:
    from concourse.bass_utils import run_bass_kernel_spmd

    in_maps = _prep_inputs(**inputs)
    nc = _get_nc()
    _STATE["in_maps"] = in_maps
    res = run_bass_kernel_spmd(nc, in_maps, list(range(len(in_maps))))
    out = np.stack(
        [r["out"].reshape(_C, _W, _W) for r in res.results]
    ).astype(np.float32)
    return out


def profile_exec_time():
    """Re-run the last inputs with NTFF tracing; returns exec time in ns."""
    from concourse.bass_utils import run_bass_kernel_spmd

    nc = _get_nc()
    in_maps = _STATE.get("in_maps")
    assert in_maps is not None, "call kernel() first"
    res = run_bass_kernel_spmd(nc, in_maps, list(range(len(in_maps))), trace=True)
    return res


# revision 60
# speedup vs baseline: 1.3937x; 1.3937x over previous
# Trainium2 Bass kernel for nn_Block_SA (dense_cnn self-attention block).
#
# Per-sample computation (C=64 channels, 64x64 spatial, N=4096 positions):
#   v   = relu(bn1(conv1x1(x)))                      # V for attention
#   s   = (x^T x) / sqrt(C)                          # [N, N] scores, Q=K=x
#   p   = softmax(s, axis=-1)
#   a   = V p^T  (a[d,n] = sum_m p[n,m] V[d,m])
#   z   = relu(bn2(depthwise3x3(a)))
#   out = bn3(conv1x1(z)) + x
#
# Distribution: batch B=8, one sample per NeuronCore (data parallel, no
# collectives). BN params are folded into conv weights on the host.
#
# On-chip algorithm (per core):
#   - Scores are computed TRANSPOSED: sT[m, n] tiles via matmul(lhsT=x[:,mtile],
#     rhs=x[:,nchunk]) so softmax's sum over m becomes a matmul reduction.
#     The K=64 contraction uses only half the PE array, so score matmuls are
#     row-packed two-at-a-time with tile_position (x duplicated on partitions
#     64-127) for ~2x PE throughput.
#   - Score/attention matmul operands are bf16 (PSUM accumulation stays fp32).
#     fp32-mode matmuls trip the HW activity throttle (util limit 0.5 for
#     ~half the run in the fp32r version); bf16 avoids it. Accuracy loss is
#     ~0.5% rel, well under the 2e-2 gate.
#   - exp() without max subtraction (scores/8 are small; fp32 is safe), in
#     batches of 3 m-tiles (3 PSUM banks) to amortize the ACT engine's fixed
#     per-instruction access latency. The scalar engine is the bottleneck
#     (~134us of exp at 1 elem/cycle/partition @1.2GHz) and runs ~97% busy
#     in steady state.
#   - The denominator sum_m exp(sT[m,n]) is obtained for free by augmenting
#     V^T with a ones column (row 64 of the AV accumulator).
#   - AV accumulates over all 32 m-tiles into one PSUM bank as serial K=128
#     matmuls: same-context streams tail-overlap (start-to-start ~215ns),
#     which beats row-packing them (two busy contexts cannot hide the fixed
#     ~173ns SBUF access latency).
#   - Normalization: 1/den via the fast custom-DVE reciprocal (18 bits; its
#     input must be partition-0-aligned — custom-DVE ops break on shifted
#     partition bases), broadcast across partitions on the (mostly idle)
#     Pool engine, multiply on DVE.
#   - Software-pipelined emission: each group's AV matmuls are emitted TWO
#     groups late, and each chunk's depthwise/conv3 are deferred into the
#     next chunk's group loop, so the in-order PE queue always has score
#     work while exp/normalize chains run on other engines.
#   - Depthwise 3x3 runs on the DVE as 9 scalar_tensor_tensor accumulation
#     taps over shifted 2-D window views of y (per-channel weights are
#     per-partition scalars). The LAST chunk's taps run as PE diag-weight
#     matmuls instead (split PE/DVE) to shorten the serial tail.
#   - conv3 + bias via augmented ones row; residual add in fp32; DMA out.
#   - bf16 ones rows arrive via DMA (single-partition wide memsets cost
#     ~3.5us on an engine).

import numpy as np

_EPS = 1e-5
_C = 64
_CP1 = 65
_N = 4096
_CH = 512          # free-dim chunk (one PSUM bank of fp32)
_NCH = _N // _CH   # 8 chunks
_MT = 128          # m-tile (partition dim of transposed score tiles)
_NMT = _N // _MT   # 32 m-tiles
_W = 64            # image width
_GROUPS = [3] * 10 + [2]   # m-tiles per exp batch (3 PSUM banks per batch)
_NCONST = 138 + 9 * 64 + 9  # w1aug | w3aug | w2p | b2p | diags | -w2p

_STATE = {}


def _build_program(reps=1):
    import concourse.bacc as bacc
    import concourse.tile as tile
    from concourse import mybir

    F32 = mybir.dt.float32
    F32R = mybir.dt.float32r
    BF16 = mybir.dt.bfloat16
    U32 = mybir.dt.uint32
    AF = mybir.ActivationFunctionType
    ALU = mybir.AluOpType
    ONE_BITS = 0x3F800000

    nc = bacc.Bacc(None)

    xd = nc.dram_tensor("x", [_C, _N], F32, kind="ExternalInput")
    # packed weights -> one DMA: cols 0:64 w1aug, 64:128 w3aug,
    # 128:137 w2p (rows 0:64), 137 b2p, 138:714 diag(w2p[:,k]) k=0..8
    cd = nc.dram_tensor("consts", [_CP1, _NCONST], F32, kind="ExternalInput")
    onesd = nc.dram_tensor("ones_bf", [1, _N], BF16, kind="ExternalInput")
    outd = nc.dram_tensor("out", [_C, _N], F32, kind="ExternalOutput")

    with tile.TileContext(nc) as tc:
        with (
            tc.tile_pool(name="persist", bufs=1) as pp,
            tc.tile_pool(name="small", bufs=2) as sp,
            tc.tile_pool(name="pt_pool", bufs=4) as ptp,
            tc.tile_pool(name="ps_pool", bufs=2, space="PSUM") as psp,
            tc.tile_pool(name="po_pool", bufs=1, space="PSUM") as pop,
            tc.tile_pool(name="aux_pool", bufs=1, space="PSUM") as auxp,
        ):
            def emit_all():
                # ---- input staging. x is DMA'd from HBM once (fp32, kept for
                # the residual), cast to bf16 on DVE/Pool, and the bf16 copy is
                # duplicated to partitions 64:128 by SBUF-to-SBUF DMA so score
                # matmuls can row-pack.
                xo = pp.tile([_C, _N], F32, name="xo", tag="xo")
                xa = pp.tile([_CP1, _N], BF16, name="xa", tag="xa")
                xb2 = pp.tile([_MT, _N], BF16, name="xb2", tag="xb2")
                lo = 0
                for s, w in enumerate([512, 512, 1024, 1024, 1024]):
                    sl = slice(lo, lo + w)
                    lo += w
                    nc.sync.dma_start(xo[:, sl], xd[:, sl])
                    eng = nc.vector if s < 4 else nc.gpsimd
                    eng.tensor_copy(xa[0:_C, sl], xo[:, sl])
                    nc.sync.dma_start(xb2[_C:_MT, sl], xa[0:_C, sl])
                # bf16 ones rows come in via DMA (a single-partition 4096-wide
                # memset costs ~3.5us on an engine; the DMA is ~free)
                nc.sync.dma_start(xa[_C:_CP1, :], onesd[:])

                # PE p-state warm-up: the tensor engine needs ~3us of
                # continuous execution to reach full clock, and it is idle
                # during the startup DMA wait anyway. Stream a few dummy
                # matmuls from a memset tile so chunk 0's real matmuls run
                # at full speed.
                wu = pp.tile([_C, _CH], BF16, name="wu", tag="wu")
                nc.vector.memset(wu[:], 0.5)
                wps = auxp.tile([_MT, _CH], F32, name="wps", tag="aux")
                for _ in range(10):
                    nc.tensor.matmul(
                        wps[0:_C, :], lhsT=wu[:, 0:_C], rhs=wu[:],
                        start=True, stop=True,
                    )

                cs = pp.tile([_CP1, _NCONST], F32, name="cs", tag="cs")
                nc.scalar.dma_start(cs[:], cd[:])
                w2s = cs[0:_C, 128:137]
                b2s = cs[0:_C, 137:138]
                w2n = cs[0:_C, 714:723]

                # rounded weight copies (bf16 everywhere on the PE)
                w1b = pp.tile([_CP1, _C], BF16, name="w1b", tag="w1b")
                nc.vector.tensor_copy(w1b[:], cs[:, 0:64])
                w3b = pp.tile([_CP1, _C], BF16, name="w3b", tag="w3b")
                nc.vector.tensor_copy(w3b[:], cs[:, 64:128])
                dgb = pp.tile([_C, 9 * _C], BF16, name="dgb", tag="dgb")
                nc.gpsimd.tensor_copy(dgb[:], cs[0:_C, 138:138 + 9 * _C])

                # V^T blocks: per m-tile a [128, 65] block (col 64 = ones)
                vt = pp.tile([_MT, _NMT * _CP1], BF16, name="vt", tag="vt")
                vt3 = vt.rearrange("p (t c) -> p t c", c=_CP1)
                nc.gpsimd.memset(vt3[:, :, _C:_CP1], 1.0)

                # normalized attention output (bf16: feeds the PE depthwise).
                # One zeroed pad row of 64 on each side so flat row-spanning
                # shifted reads stay in bounds.
                yrp = pp.tile([_C, _N + 2 * _W], BF16, name="yrp", tag="yrp")
                nc.gpsimd.memset(yrp[:, 0:_W], 0.0)
                nc.gpsimd.memset(yrp[:, _W + _N : _N + 2 * _W], 0.0)
                yr = yrp[:, _W : _W + _N]
                # post-depthwise activations (+ones row) feeding conv3
                zr = pp.tile([_CP1, _N], BF16, name="zr", tag="zr")
                nc.sync.dma_start(zr[_C:_CP1, :], onesd[:])
                zrv = zr[0:_C, :].rearrange("c (h w) -> c h w", w=_W)

                # ---- V^T groups: emitted lazily (interleaved into chunk 0's
                # group loop) so the cold PE isn't blocked on them at startup.
                # relu on DVE (not ACT) so the scalar engine runs Exp only.
                _vt_emitted = [0]

                def emit_vt_groups(need_mtiles):
                    while _vt_emitted[0] * 8 < need_mtiles:
                        g = _vt_emitted[0]
                        vps = auxp.tile([_MT, 8 * _C], F32, name="vps", tag="aux")
                        for j in range(8):
                            m = 8 * g + j
                            nc.tensor.matmul(
                                vps[:, _C * j : _C * (j + 1)],
                                lhsT=xa[:, _MT * m : _MT * (m + 1)],
                                rhs=w1b[:],
                                start=True,
                                stop=True,
                            )
                        nc.vector.tensor_relu(
                            vt3[:, 8 * g : 8 * (g + 1), 0:_C],
                            vps[:].rearrange("p (t c) -> p t c", c=_C),
                        )
                        _vt_emitted[0] += 1

                # ---- depthwise 3x3 on the (otherwise idle) GpSimd engine:
                # per-channel tap weights are per-partition scalars, so each
                # tap is one scalar_tensor_tensor accumulation over a shifted
                # 2-D window view of y. No PE involvement, no wrap fix-ups.
                yrp3 = yrp.rearrange("c (h w) -> c h w", w=_W)  # row i = y row i-1

                def emit_dw(h0, h1):
                    nh = h1 - h0
                    za = sp.tile([_C, nh * _W], F32, name="za", tag="za", bufs=2)
                    za3 = za.rearrange("c (h w) -> c h w", w=_W)
                    # init with the center tap + bias: z = y*w4 + b
                    nc.vector.tensor_scalar(
                        za3[:], yrp3[:, h0 + 1 : h1 + 1, :], w2s[:, 4:5], b2s,
                        op0=ALU.mult, op1=ALU.add,
                    )
                    for k in [0, 1, 2, 3, 5, 6, 7, 8]:
                        dy, dx = k // 3 - 1, k % 3 - 1
                        hh0, hh1 = max(h0, -dy), min(h1, _W - dy)
                        if hh1 <= hh0:
                            continue
                        x0, x1 = max(0, -dx), _W - max(0, dx)
                        dst = za3[:, hh0 - h0 : hh1 - h0, x0:x1]
                        src = yrp3[:, hh0 + dy + 1 : hh1 + dy + 1, x0 + dx : x1 + dx]
                        nc.vector.scalar_tensor_tensor(
                            dst, src, w2s[:, k : k + 1], dst,
                            op0=ALU.mult, op1=ALU.add,
                        )
                    nc.vector.tensor_scalar(
                        zrv[:, h0:h1, :], za3[:], 0.0, 0.0, op0=ALU.max, op1=ALU.max
                    )

                def emit_dw_pe(h0, h1):
                    # PE variant (diag-weight matmuls over clipped 2-D window
                    # views): used for the LAST chunk, where the serial DVE
                    # chain would sit alone in the tail while the PE idles.
                    nh = h1 - h0
                    dwp = auxp.tile([_C, nh * _W], F32, name="dwp", tag="aux")
                    dwp3 = dwp.rearrange("c (h w) -> c h w", w=_W)
                    taps = []
                    for k in [4, 0, 1, 2, 3, 5, 6, 7, 8]:
                        dy, dx = k // 3 - 1, k % 3 - 1
                        hh0, hh1 = max(h0, -dy), min(h1, _W - dy)
                        if hh1 <= hh0:
                            continue
                        x0, x1 = max(0, -dx), _W - max(0, dx)
                        taps.append((k, hh0, hh1, x0, x1, dy, dx))
                    for i, (k, hh0, hh1, x0, x1, dy, dx) in enumerate(taps):
                        nc.tensor.matmul(
                            dwp3[:, hh0 - h0 : hh1 - h0, x0:x1],
                            lhsT=dgb[:, _C * k : _C * (k + 1)],
                            rhs=yrp3[:, hh0 + dy + 1 : hh1 + dy + 1,
                                     x0 + dx : x1 + dx],
                            start=(i == 0),
                            stop=(i == len(taps) - 1),
                            skip_group_check=True,
                        )
                    nc.vector.tensor_scalar(
                        zrv[:, h0:h1, :], dwp3[:], b2s, 0.0,
                        op0=ALU.add, op1=ALU.max,
                    )

                def emit_conv3(c):
                    # conv3 (+bias via ones row) + residual + store
                    pc = auxp.tile([_C, _CH], F32, name="pc", tag="aux")
                    nc.tensor.matmul(
                        pc[:],
                        lhsT=w3b[:],
                        rhs=zr[:, _CH * c : _CH * (c + 1)],
                        start=True,
                        stop=True,
                    )
                    outt = sp.tile([_C, _CH], F32, name="outt", tag="outt", bufs=2)
                    nc.vector.tensor_tensor(
                        outt[:], pc[:], xo[:, _CH * c : _CH * (c + 1)], op=ALU.add
                    )
                    nc.sync.dma_start(outd[:, _CH * c : _CH * (c + 1)], outt[:])

                # ---- main fused-attention loop over n-chunks ----
                # Deferred depthwise/conv3 closures, popped between the NEXT
                # chunk's score groups (keeps the in-order PE queue stall-free).
                pending = []
                # Software-pipelined emission: exp feeds an AV-group QUEUE
                # drained TWO groups behind the scores — the in-order PE queue
                # then always has score groups ready to run while the scalar
                # engine computes exp. The queue persists across chunk
                # boundaries; each chunk's normalize is emitted right after
                # its LAST AV group pops (so the accumulator is fully written
                # in emission order before being read).
                av_q = []
                _AV_DELAY = 2

                def emit_normalize(po, ci):
                    # normalize: y = u[0:64] * (1/u[64]). Custom-DVE ops need a
                    # partition-0-aligned source, so first stage the den row to
                    # partition 0 with a plain (shift-capable) DVE copy, then
                    # the fast reciprocal; partition-broadcast on Pool (PE and
                    # ACT stay out of this chain entirely).
                    dsb = sp.tile([1, _CH], F32, name="dsb", tag="dsb", bufs=2)
                    nc.vector.tensor_copy(dsb[:], po[_C : _C + 1, :])
                    invf = sp.tile([1, _CH], F32, name="invf", tag="invf", bufs=2)
                    nc.vector.reciprocal_approx_fast(out=invf[:], in_=dsb[:])
                    bcps = sp.tile([_C, _CH], F32, name="bcps", tag="bcps", bufs=2)
                    nc.gpsimd.partition_broadcast(bcps[:], invf[:])
                    nc.vector.tensor_tensor(
                        yr[:, _CH * ci : _CH * (ci + 1)], po[0:_C, :], bcps[:],
                        op=ALU.mult,
                    )
                    # queue this chunk's depthwise (and finish chunk ci-1:
                    # its boundary row needed this chunk's y). The last chunk's
                    # depthwise runs on the PE (tail latency).
                    if ci == _NCH - 1:
                        # split the tail depthwise across PE and DVE so the
                        # two halves run concurrently
                        def dwtail(ci=ci):
                            emit_dw_pe(8 * ci, 8 * ci + 4)
                            emit_dw(8 * ci + 4, 8 * ci + 7)
                        pending.append(dwtail)
                        dwf = emit_dw_pe
                    else:
                        dwf = emit_dw
                        pending.append(lambda f=dwf, ci=ci: f(8 * ci, 8 * ci + 7))
                    if ci >= 1:
                        def fin(ci=ci, f=dwf):
                            f(8 * ci - 1, 8 * ci)
                            emit_conv3(ci - 1)
                        pending.append(fin)

                def pop_av():
                    emit, need, fin_ci_po = av_q.pop(0)
                    # V^T blocks are emitted just-in-time for the AV group
                    # that consumes them (eager emission would delay the
                    # early score groups and starve exp)
                    if need is not None:
                        emit_vt_groups(need)
                    emit()
                    if fin_ci_po is not None:
                        emit_normalize(*fin_ci_po)

                for ci in range(_NCH):
                    po = pop.tile([_MT, _CH], F32, name="po", tag="po")
                    m = 0
                    # chunk 0 leads with small groups so the scalar engine's exp
                    # stream starts as soon as the first score tile exists
                    groups = ([1, 2] + [3] * 9 + [2]) if ci == 0 else _GROUPS
                    for gi, msz in enumerate(groups):
                        ps = psp.tile([_MT, _CH * msz], F32, name="ps", tag="ps")
                        for j in range(msz):
                            mt = m + j
                            if mt % 2 == 0:
                                src, rows, tp = xa, slice(0, _C), (0, 0)
                            else:
                                src, rows, tp = xb2, slice(_C, _MT), (_C, 0)
                            nc.tensor.matmul(
                                ps[:, _CH * j : _CH * (j + 1)],
                                lhsT=src[rows, _MT * mt : _MT * (mt + 1)],
                                rhs=src[rows, _CH * ci : _CH * (ci + 1)],
                                start=True,
                                stop=True,
                                tile_position=tp,
                            )
                        pt = ptp.tile([_MT, _CH * msz], BF16, name="pt", tag="pt")
                        nc.scalar.activation(pt[:], ps[:], AF.Exp, scale=0.125)

                        def av_group(po=po, pt=pt, m=m, msz=msz):
                            for j in range(msz):
                                nc.tensor.matmul(
                                    po[0:_CP1, :],
                                    lhsT=vt[:, _CP1 * (m + j) : _CP1 * (m + j + 1)],
                                    rhs=pt[:, _CH * j : _CH * (j + 1)],
                                    start=(m + j == 0),
                                    stop=(m + j == _NMT - 1),
                                    skip_group_check=True,
                                )

                        last = m + msz == _NMT
                        av_q.append((av_group, (m + msz) if ci == 0 else None,
                                     (po, ci) if last else None))
                        while len(av_q) > _AV_DELAY:
                            pop_av()
                        m += msz
                        if gi in (5, 10) and pending:
                            pending.pop(0)()
                while av_q:
                    pop_av()
                for f in pending:
                    f()
                emit_dw_pe(_N // _W - 1, _N // _W)  # last row (no dy=+1 tap)
                emit_conv3(_NCH - 1)

            if reps == 1:
                emit_all()
            else:
                with tc.For_i(0, reps, 1):
                    emit_all()

    nc.finalize()
    return nc


def _get_nc():
    if "nc" not in _STATE:
        _STATE["nc"] = _build_program()
    return _STATE["nc"]


def _prep_inputs(x, w1, bn1_g, bn1_b, bn1_m, bn1_v,
                 w2, bn2_g, bn2_b, bn2_m, bn2_v,
                 w3, bn3_g, bn3_b, bn3_m, bn3_v):
    f32 = np.float32
    x = np.asarray(x, f32)
    inv1 = np.asarray(bn1_g, f32) / np.sqrt(np.asarray(bn1_v, f32) + _EPS)
    w1p = np.asarray(w1, f32)[:, :, 0, 0] * inv1[:, None]
    b1p = np.asarray(bn1_b, f32) - np.asarray(bn1_m, f32) * inv1
    w1aug = np.concatenate([w1p.T, b1p[None, :]], axis=0)

    inv2 = np.asarray(bn2_g, f32) / np.sqrt(np.asarray(bn2_v, f32) + _EPS)
    w2p = np.asarray(w2, f32)[:, 0].reshape(_C, 9) * inv2[:, None]
    b2p = (np.asarray(bn2_b, f32) - np.asarray(bn2_m, f32) * inv2)[:, None]

    inv3 = np.asarray(bn3_g, f32) / np.sqrt(np.asarray(bn3_v, f32) + _EPS)
    w3p = np.asarray(w3, f32)[:, :, 0, 0] * inv3[:, None]
    b3p = np.asarray(bn3_b, f32) - np.asarray(bn3_m, f32) * inv3
    w3aug = np.concatenate([w3p.T, b3p[None, :]], axis=0)

    consts = np.zeros((_CP1, _NCONST), f32)
    consts[:, 0:64] = w1aug
    consts[:, 64:128] = w3aug
    consts[0:_C, 128:137] = w2p
    consts[0:_C, 137:138] = b2p
    for k in range(9):
        consts[0:_C, 138 + _C * k : 138 + _C * (k + 1)] = np.diag(w2p[:, k])
    consts[0:_C, 714:723] = -w2p

    import ml_dtypes
    ones_bf = np.ones((1, _N), dtype=ml_dtypes.bfloat16)
    B = x.shape[0]
    in_maps = []
    for i in range(B):
        in_maps.append({
            "x": np.ascontiguousarray(x[i].reshape(_C, _N)),
            "consts": consts,
            "ones_bf": ones_bf,
        })
    return in_maps


def kernel(**inputs) -> np.ndarray:
    from concourse.bass_utils import run_bass_kernel_spmd

    in_maps = _prep_inputs(**inputs)
    nc = _get_nc()
    _STATE["in_maps"] = in_maps
    res = run_bass_kernel_spmd(nc, in_maps, list(range(len(in_maps))))
    out = np.stack(
        [r["out"].reshape(_C, _W, _W) for r in res.results]
    ).astype(np.float32)
    return out


def profile_exec_time():
    """Re-run the last inputs with NTFF tracing; returns exec time in ns."""
    from concourse.bass_utils import run_bass_kernel_spmd

    nc = _get_nc()
    in_maps = _STATE.get("in_maps")
    assert in_maps is not None, "call kernel() first"
    res = run_bass_kernel_spmd(nc, in_maps, list(range(len(in_maps))), trace=True)
    return res
